# revision 1
# baseline (speedup 1.0000x reference)
"""Multi-head attention (GQA, 32 q-heads / 8 kv-heads, S=2048, H=4096) on 8
Trainium2 NeuronCores.

Sharding: tensor-parallel across heads. Core c owns kv-head c and q-heads
4c..4c+3 (Wq/Wk/Wv column-sharded, Wo row-sharded). Each core computes a
partial output [S, H]; the host sums the 8 partials.

Per-core dataflow (everything bf16 into the PE, fp32 accumulation):
  A) qT/kT/vT = W.T @ hiddenT  (weights stationary, hiddenT moving)
     + RoPE applied in the transposed [hd, s] layout
     + vT transposed back to natural v[s, hd] via PE-transpose
  B) per q-head: scoresT[j,i] = kT.T @ qT  ->  E = exp(scale*scoresT)
     denom[i] = onesT @ E (PE),  attnT[d,i] = v.T @ E, normalized on DVE
  C) partial_out[s,:] = attnT.T @ Wo_c  (attnT stationary, Wo moving)
"""

import math
import os
import sys

if os.path.isdir("/opt/trn_rl_repo") and "/opt/trn_rl_repo" not in sys.path:
    sys.path.insert(0, "/opt/trn_rl_repo")

import numpy as np
import ml_dtypes

import concourse.bacc as bacc
import concourse.mybir as mybir
from concourse import tile
from concourse.bass_utils import run_bass_kernel_spmd

BF16 = mybir.dt.bfloat16
F32 = mybir.dt.float32
NPBF16 = ml_dtypes.bfloat16

S = 2048
H = 4096
HD = 128
NH = 32
NKV = 8
N_CORES = 8
QH = NH // N_CORES          # q-heads per core = 4
F = QH * HD                 # q feature columns per core = 512
KT = H // 128               # contraction tiles for the projections = 32
ST = S // 128               # 128-row tiles along S = 16
SG = S // 512               # 512-wide groups along S = 4
SCALE = 1.0 / math.sqrt(HD)

_BUILT = {}


def _build(mode: str):
    masked = mode == "generic"
    nc = bacc.Bacc(None, target_bir_lowering=False)

    hT = nc.declare_dram_parameter("hT", [H, S], BF16, isOutput=False)
    wqkv = nc.declare_dram_parameter("wqkv", [H, F + 2 * HD], BF16, isOutput=False)
    wo = nc.declare_dram_parameter("wo", [F, H], BF16, isOutput=False)
    cosT = nc.declare_dram_parameter("cosT", [HD, S], F32, isOutput=False)
    sinTe = nc.declare_dram_parameter("sinTe", [HD, S], F32, isOutput=False)
    eye = nc.declare_dram_parameter("eye", [128, 128], BF16, isOutput=False)
    if masked:
        maskT = nc.declare_dram_parameter("maskT", [S, S], F32, isOutput=False)
    if mode == "causal":
        # four 0/1 diagonal-tile patterns, stacked [4*128, 512]
        m01 = nc.declare_dram_parameter("m01", [4 * 128, 512], BF16, isOutput=False)
    out = nc.declare_dram_parameter("out", [S, H], F32, isOutput=True)

    FW = F + 2 * HD  # 768 weight columns per contraction tile

    with tile.TileContext(nc) as tc:
        with tc.tile_pool(name="persist", bufs=1) as pp:
            # persistent SBUF tensors
            cos_sb = pp.tile([HD, S], F32, tag="cos")
            sin_sb = pp.tile([HD, S], F32, tag="sin")
            eye_sb = pp.tile([128, 128], BF16, tag="eye")
            ones_sb = pp.tile([128, 128], BF16, tag="ones")
            qT_sb = [pp.tile([HD, S], BF16, tag=f"qT{h}", name=f"qT{h}") for h in range(QH)]
            kT_sb = pp.tile([HD, S], BF16, tag="kT")
            v_sb = pp.tile([128, ST * HD], BF16, tag="v")  # block jt: v[jt*128:(jt+1)*128, :]
            aT_sb = [pp.tile([HD, S], BF16, tag=f"aT{h}", name=f"aT{h}") for h in range(QH)]
            # RoPE scratch lives in the persistent pool so phase-B tiles
            # never alias it (an aliased WAR here would serialize phase B
            # behind the whole RoPE tail)
            vt_t = pp.tile([128, 512], BF16, tag="vt")
            ev_t = {
                f: pp.tile([128, 512], F32, tag=f"ev{f}", name=f"ev{f}")
                for f in (QH, 0, 1, 2, 3)
            }
            t1_t = pp.tile([128, 512], F32, tag="t1")
            t2_t = pp.tile([128, 512], F32, tag="t2")
            if mode == "causal":
                m01_sb = pp.tile([128, 4 * 512], BF16, tag="m01")

            nc.gpsimd.memset(ones_sb[:], 1.0)

            # ---------------- Phase A: projections + RoPE ----------------
            with (
                tc.tile_pool(name="phA_sb", bufs=1) as pa,
                tc.tile_pool(name="phA_h", bufs=1) as pah,
                tc.tile_pool(name="phA_ps", bufs=1, space="PSUM") as pap,
                tc.tile_pool(name="phA_pst", bufs=1, space="PSUM") as papt,
            ):
                w_sb = pa.tile([128, KT * FW], BF16, tag="wqkv")
                w_view = w_sb[:].rearrange("p (a f) -> p a f", a=KT)
                w_src = wqkv[:].rearrange("(a p) f -> p a f", p=128)
                CH = 4  # ktiles per DMA chunk
                for sg in range(SG):
                    hc = pah.tile([128, KT * 512], BF16, tag="hc")
                    h_view = hc[:].rearrange("p (a s) -> p a s", a=KT)
                    h_src = hT[:, sg * 512:(sg + 1) * 512].rearrange(
                        "(a p) s -> p a s", p=128
                    )
                    # interleave weight/hidden chunk loads so the PE can
                    # start as soon as the first k-tiles land
                    bounds = ([0, 1, 2, 4] if sg == 0 else []) + list(
                        range(4 if sg == 0 else 0, KT, CH)
                    )[1 if sg == 0 else 0:]
                    bounds = sorted(set(bounds + [KT]))
                    for lo, hi in zip(bounds[:-1], bounds[1:]):
                        csl = slice(lo, hi)
                        if sg == 0:
                            nc.sync.dma_start(w_view[:, csl, :], w_src[:, csl, :])
                        nc.sync.dma_start(h_view[:, csl, :], h_src[:, csl, :])
                        if sg == 0 and lo == 28:
                            # needed only ~40us in; keep off the queue head
                            nc.sync.dma_start(eye_sb[:], eye[:])
                            nc.sync.dma_start(cos_sb[:], cosT[:])
                            nc.sync.dma_start(sin_sb[:], sinTe[:])
                            if mode == "causal":
                                nc.sync.dma_start(
                                    m01_sb[:].rearrange("p (a i) -> p a i", a=4),
                                    m01[:].rearrange("(a p) i -> p a i", p=128),
                                )
                    pss = [
                        pap.tile([128, 512], F32, tag=f"proj{f}", name=f"proj{f}",
                                 bufs=2 if f == 0 else 1)
                        for f in range(QH + 2)
                    ]
                    for k in range(KT):
                        for f in range(QH + 2):
                            nc.tensor.matmul(
                                pss[f][:],
                                w_sb[:, k * FW + f * 128:k * FW + (f + 1) * 128],
                                hc[:, k * 512:(k + 1) * 512],
                                start=(k == 0),
                                stop=(k == KT - 1),
                            )
                    # Evict all six PSUM groups with plain copies first (banks
                    # free fast, split across DVE/ACT); v-transposes and the
                    # k RoPE come before the q RoPEs so phase B can start.
                    sl = slice(sg * 512, (sg + 1) * 512)
                    vt = vt_t
                    nc.vector.tensor_copy(vt[:], pss[QH + 1][:])
                    for f in (QH, 0, 1, 2, 3):
                        ev = ev_t[f]
                        if f in (0, 2):
                            nc.vector.tensor_copy(ev[:], pss[f][:])
                        else:
                            nc.scalar.copy(ev[:], pss[f][:])
                    for b in range(4):
                        jt = sg * 4 + b
                        pst = papt.tile([128, 128], BF16, tag="vtr")
                        nc.tensor.transpose(
                            pst[:], vt[:, b * 128:(b + 1) * 128], eye_sb[:]
                        )
                        cp = nc.vector.tensor_copy if b % 2 == 0 else nc.scalar.copy
                        cp(v_sb[:, jt * HD:(jt + 1) * HD], pst[:])
                    for f in (QH, 0, 1, 2, 3):
                        # RoPE: out[d] = x[d]*cos[d] + x[(d+64)%128]*sinTe[d]
                        ev = ev_t[f]
                        dest = (qT_sb[f] if f < QH else kT_sb)[:, sl]
                        t1 = t1_t
                        t2 = t2_t
                        nc.vector.tensor_mul(t1[:], ev[:], cos_sb[:, sl])
                        # sin table is host-rolled by 64 rows so both SBUF
                        # inputs share a base partition (walrus constraint)
                        nc.vector.tensor_mul(
                            t2[0:64, :], ev[64:128, :], sin_sb[64:128, sl]
                        )
                        nc.vector.tensor_mul(
                            t2[64:128, :], ev[0:64, :], sin_sb[0:64, sl]
                        )
                        nc.vector.tensor_add(dest, t1[:], t2[:])

            # ---------------- Phase B: attention per head ----------------
            with tc.tile_pool(name="late", bufs=1) as pl:
              # Wo is only needed in phase C; issue its load here so the
              # transfer hides under phase B compute
              wo_sb = pl.tile([128, QH * H], BF16, tag="wo")
              nc.sync.dma_start(
                  wo_sb[:].rearrange("p (a o) -> p a o", a=QH),
                  wo[:].rearrange("(a p) o -> p a o", p=128),
              )
              with (
                tc.tile_pool(name="phB_E", bufs=18) as pe_pool,
                tc.tile_pool(name="phB_tmp", bufs=3) as pbt,
                tc.tile_pool(name="phB_m", bufs=3) as pbm,
                tc.tile_pool(name="phB_s", bufs=4, space="PSUM") as pbs,
                tc.tile_pool(name="phB_acc", bufs=2, space="PSUM") as pba,
              ):
                  for h in range(QH):
                      qh = qT_sb[h]
                      et = [pe_pool.tile([128, S], BF16, tag="E", name=f"E{h}_{j}") for j in range(ST)]
                      for ig in range(SG):
                          isl = slice(ig * 512, (ig + 1) * 512)
                          # causal: key tiles past this query block contribute
                          # exactly zero -- skip them entirely
                          jts = list(range(4 * ig + 4)) if mode == "causal" else list(range(ST))
                          for jt in jts:
                              sps = pbs.tile([128, 512], F32, tag="s")
                              nc.tensor.matmul(
                                  sps[:],
                                  kT_sb[:, jt * 128:(jt + 1) * 128],
                                  qh[:, isl],
                                  start=True,
                                  stop=True,
                              )
                              if masked:
                                  # host pre-scales maskT by sqrt(HD):
                                  # exp(SCALE*(scores + maskT)) == softmax logits
                                  mt = pbm.tile([128, 512], F32, tag="mT")
                                  nc.sync.dma_start(
                                      mt[:], maskT[jt * 128:(jt + 1) * 128, isl]
                                  )
                                  sm = pbm.tile([128, 512], F32, tag="sm")
                                  nc.vector.tensor_add(sm[:], sps[:], mt[:])
                                  nc.scalar.activation(
                                      et[jt][:, isl], sm[:],
                                      mybir.ActivationFunctionType.Exp,
                                      scale=SCALE,
                                  )
                              elif mode == "causal" and jt >= 4 * ig:
                                  # diagonal tile: exp then zero the j>i part
                                  p = jt - 4 * ig
                                  etmp = pbm.tile([128, 512], BF16, tag="etmp")
                                  nc.scalar.activation(
                                      etmp[:], sps[:],
                                      mybir.ActivationFunctionType.Exp,
                                      scale=SCALE,
                                  )
                                  nc.vector.tensor_mul(
                                      et[jt][:, isl], etmp[:],
                                      m01_sb[:, p * 512:(p + 1) * 512],
                                  )
                              else:
                                  nc.scalar.activation(
                                      et[jt][:, isl], sps[:],
                                      mybir.ActivationFunctionType.Exp,
                                      scale=SCALE,
                                  )
                          den = pba.tile([128, 512], F32, tag="den")
                          for jt in jts:
                              nc.tensor.matmul(
                                  den[:], ones_sb[:], et[jt][:, isl],
                                  start=(jt == jts[0]), stop=(jt == jts[-1]),
                              )
                          pv = pba.tile([128, 512], F32, tag="pv")
                          for jt in jts:
                              nc.tensor.matmul(
                                  pv[:], v_sb[:, jt * HD:(jt + 1) * HD], et[jt][:, isl],
                                  start=(jt == jts[0]), stop=(jt == jts[-1]),
                              )
                          rc = pbt.tile([128, 512], F32, tag="rc")
                          nc.vector.reciprocal_approx_fast(rc[:], den[:])
                          nc.vector.tensor_mul(aT_sb[h][:, isl], pv[:], rc[:])

              # ---------------- Phase C: output projection ----------------
              with (
                  tc.tile_pool(name="phC_sb", bufs=1) as pc,
                  tc.tile_pool(name="phC_o", bufs=6) as pco,
                  tc.tile_pool(name="phC_ps", bufs=6, space="PSUM") as pcp,
              ):
                  for st in range(ST):
                      ssl = slice(st * 128, (st + 1) * 128)
                      for ho in range(H // 512):
                          po = pcp.tile([128, 512], F32, tag="o")
                          for f4 in range(QH):
                              nc.tensor.matmul(
                                  po[:],
                                  aT_sb[f4][:, ssl],
                                  wo_sb[:, f4 * H + ho * 512:f4 * H + (ho + 1) * 512],
                                  start=(f4 == 0),
                                  stop=(f4 == QH - 1),
                              )
                          ob = pco.tile([128, 512], F32, tag="ob")
                          if ho % 2 == 0:
                              nc.scalar.copy(ob[:], po[:])
                          else:
                              nc.vector.tensor_copy(ob[:], po[:])
                          nc.sync.dma_start(
                              out[ssl, ho * 512:(ho + 1) * 512], ob[:]
                          )

    nc.finalize()
    return nc


def _get_kernel(mode: str):
    if mode not in _BUILT:
        _BUILT[mode] = _build(mode)
    return _BUILT[mode]


def _detect_mode(mask2d):
    if not np.any(mask2d):
        return "nomask"
    neg = mask2d[0, 1]
    if neg <= -1e4 and np.array_equal(
        mask2d, np.triu(np.full((S, S), neg, mask2d.dtype), k=1)
    ):
        return "causal"
    return "generic"


def kernel(hidden_states, position_ids, attention_mask, cos, sin, Wq, Wk, Wv, Wo,
           _collect_exec_info=None):
    hidden_states = np.asarray(hidden_states)
    attention_mask = np.asarray(attention_mask)
    cos = np.asarray(cos)
    sin = np.asarray(sin)
    Wq, Wk, Wv, Wo = (np.asarray(a) for a in (Wq, Wk, Wv, Wo))

    mode = _detect_mode(attention_mask[0, 0])
    masked = mode == "generic"
    nc = _get_kernel(mode)

    hT = np.ascontiguousarray(hidden_states[0].T).astype(NPBF16)
    cosT = np.ascontiguousarray(cos[0].T).astype(np.float32)
    sinTe = np.ascontiguousarray(sin[0].T).astype(np.float32)
    sinTe[:64] = -sinTe[:64]
    sinTe = np.ascontiguousarray(np.roll(sinTe, 64, axis=0))
    eye = np.eye(128, dtype=NPBF16)
    if mode == "causal":
        jj = np.arange(128)[:, None]
        ii = np.arange(512)[None, :]
        m01 = np.concatenate(
            [(128 * p + jj <= ii).astype(NPBF16) for p in range(4)], axis=0
        )

    in_maps = []
    for c in range(N_CORES):
        wqkv = np.concatenate(
            [
                Wq[:, c * F:(c + 1) * F],
                Wk[:, c * HD:(c + 1) * HD],
                Wv[:, c * HD:(c + 1) * HD],
            ],
            axis=1,
        ).astype(NPBF16)
        m = {
            "hT": hT,
            "wqkv": wqkv,
            "wo": Wo[c * F:(c + 1) * F, :].astype(NPBF16),
            "cosT": cosT,
            "sinTe": sinTe,
            "eye": eye,
        }
        if masked:
            m["maskT"] = (
                np.ascontiguousarray(attention_mask[0, 0].T).astype(np.float32)
                * math.sqrt(HD)
            )
        if mode == "causal":
            m["m01"] = m01
        in_maps.append(m)

    trace = _collect_exec_info is not None
    res = run_bass_kernel_spmd(nc, in_maps, list(range(N_CORES)), trace=trace)
    if trace:
        _collect_exec_info["exec_time_ns"] = res.exec_time_ns
        _collect_exec_info["results"] = res

    acc = res.results[0]["out"].astype(np.float64)
    for c in range(1, N_CORES):
        acc += res.results[c]["out"].astype(np.float64)
    return acc.astype(np.float32)[None, :, :]



# revision 2
# speedup vs baseline: 1.0987x; 1.0987x over previous
"""Multi-head attention (GQA, 32 q-heads / 8 kv-heads, S=2048, H=4096) on 8
Trainium2 NeuronCores.

Sharding: tensor-parallel across heads. Core c owns kv-head c and q-heads
4c..4c+3 (Wq/Wk/Wv column-sharded, Wo row-sharded). Each core computes a
partial output [S, H] in fp16; the host sums the 8 partials.

Per-core dataflow (everything bf16 into the PE, fp32 accumulation):
  A) qT/kT/vT = W.T @ hiddenT  (weights stationary, hiddenT moving)
     + RoPE applied in the transposed [hd, s] layout
     + vT transposed back to natural v[s, hd] via PE-transpose
     sg0/sg1 run k-major (DMA-paced); sg2/sg3 run f-major so PSUM
     evictions hide under the next feature group's matmuls.
  B) per (q-group ig, head h): scoresT[j,i] = kT.T @ qT -> E = exp(...)
     denom via a bf16 add-tree over the 16 E tiles (DVE+Pool) and a
     single ones-matmul; attnT[d,i] = v.T @ E, normalized on DVE.
  C) partial_out[s,:] = attnT.T @ Wo_c, interleaved INTO phase B: the
     C-block for query-group ig-1 is issued between the scores matmuls
     of group ig so the PE stays fed while ACT drains the exps.
"""

import math
import os
import sys

if os.path.isdir("/opt/trn_rl_repo") and "/opt/trn_rl_repo" not in sys.path:
    sys.path.insert(0, "/opt/trn_rl_repo")

import numpy as np
import ml_dtypes

import concourse.bacc as bacc
import concourse.mybir as mybir
from concourse import tile
from concourse.bass_utils import run_bass_kernel_spmd

BF16 = mybir.dt.bfloat16
F16 = mybir.dt.float16
F32 = mybir.dt.float32
NPBF16 = ml_dtypes.bfloat16

S = 2048
H = 4096
HD = 128
NH = 32
NKV = 8
N_CORES = 8
QH = NH // N_CORES          # q-heads per core = 4
F = QH * HD                 # q feature columns per core = 512
KT = H // 128               # contraction tiles for the projections = 32
ST = S // 128               # 128-row tiles along S = 16
SG = S // 512               # 512-wide groups along S = 4
SCALE = 1.0 / math.sqrt(HD)

_BUILT = {}

# feature-group order: k first (phase B needs kT complete first), then v,
# then the q heads
ORDER = (4, 5, 0, 1, 2, 3)


def _build(mode: str):
    masked = mode == "generic"
    nc = bacc.Bacc(None, target_bir_lowering=False)

    hT = nc.declare_dram_parameter("hT", [H, S], BF16, isOutput=False)
    wqkv = nc.declare_dram_parameter("wqkv", [H, F + 2 * HD], BF16, isOutput=False)
    wo = nc.declare_dram_parameter("wo", [F, H], BF16, isOutput=False)
    cosT = nc.declare_dram_parameter("cosT", [HD, S], F32, isOutput=False)
    sinTe = nc.declare_dram_parameter("sinTe", [HD, S], F32, isOutput=False)
    eye = nc.declare_dram_parameter("eye", [128, 128], BF16, isOutput=False)
    if masked:
        maskT = nc.declare_dram_parameter("maskT", [S, S], F32, isOutput=False)
    if mode == "causal":
        # four 0/1 diagonal-tile patterns, stacked [4*128, 512]
        m01 = nc.declare_dram_parameter("m01", [4 * 128, 512], BF16, isOutput=False)
    out = nc.declare_dram_parameter("out", [S, H], F16, isOutput=True)

    FW = F + 2 * HD  # 768 weight columns per contraction tile

    with tile.TileContext(nc) as tc:
        with tc.tile_pool(name="persist", bufs=1) as pp:
            # persistent SBUF tensors
            cos_sb = pp.tile([HD, S], F32, tag="cos")
            sin_sb = pp.tile([HD, S], F32, tag="sin")
            eye_sb = pp.tile([128, 128], BF16, tag="eye")
            ones_sb = pp.tile([128, 128], BF16, tag="ones")
            qT_sb = [pp.tile([HD, S], BF16, tag=f"qT{h}", name=f"qT{h}") for h in range(QH)]
            kT_sb = pp.tile([HD, S], BF16, tag="kT")
            v_sb = pp.tile([128, ST * HD], BF16, tag="v")  # block jt: v[jt*128:(jt+1)*128, :]
            aT_sb = [pp.tile([HD, S], BF16, tag=f"aT{h}", name=f"aT{h}") for h in range(QH)]
            # RoPE scratch lives in the persistent pool so phase-B tiles
            # never alias it
            vt_t = pp.tile([128, 512], BF16, tag="vt")
            ev_t = {
                f: pp.tile([128, 512], F32, tag=f"ev{f}", name=f"ev{f}")
                for f in (4, 0, 1, 2, 3)
            }
            t1_t = pp.tile([128, 512], F32, tag="t1")
            t2_t = pp.tile([128, 512], F32, tag="t2")
            if mode == "causal":
                m01_sb = pp.tile([128, 4 * 512], BF16, tag="m01")

            nc.gpsimd.memset(ones_sb[:], 1.0)

            # wqkv column ranges per feature group
            def wcol(f):
                base = f * 128 if f < QH else F + (f - QH) * HD
                return base

            # ---------------- Phase A: projections + RoPE ----------------
            with (
                tc.tile_pool(name="phA_sb", bufs=1) as pa,
                tc.tile_pool(name="phA_h", bufs=2) as pah,
                tc.tile_pool(name="phA_ps", bufs=1, space="PSUM") as pap,
                tc.tile_pool(name="phA_pst", bufs=2, space="PSUM") as papt,
            ):
                w_sb = pa.tile([128, KT * FW], BF16, tag="wqkv")
                w_view = w_sb[:].rearrange("p (a f) -> p a f", a=KT)
                w_src = wqkv[:].rearrange("(a p) f -> p a f", p=128)
                CH = 4  # ktiles per DMA chunk

                def evict_f(sg, f, pss_f):
                    """PSUM -> SBUF eviction + RoPE (or v transpose) for one
                    feature group."""
                    sl = slice(sg * 512, (sg + 1) * 512)
                    if f == 5:
                        # v: copy out, then transpose 128x128 blocks back to
                        # natural [s, hd] layout
                        nc.vector.tensor_copy(vt_t[:], pss_f[:])
                        for b in range(4):
                            jt = sg * 4 + b
                            pst = papt.tile([128, 128], BF16, tag="vtr")
                            nc.tensor.transpose(
                                pst[:], vt_t[:, b * 128:(b + 1) * 128], eye_sb[:]
                            )
                            cp = nc.vector.tensor_copy if b % 2 == 0 else nc.scalar.copy
                            cp(v_sb[:, jt * HD:(jt + 1) * HD], pst[:])
                        return
                    ev = ev_t[f]
                    if f in (0, 2):
                        nc.vector.tensor_copy(ev[:], pss_f[:])
                    else:
                        nc.scalar.copy(ev[:], pss_f[:])
                    # RoPE: out[d] = x[d]*cos[d] + x[(d+64)%128]*sinTe[d]
                    dest = (qT_sb[f] if f < QH else kT_sb)[:, sl]
                    nc.vector.tensor_mul(t1_t[:], ev[:], cos_sb[:, sl])
                    # sin table is host-rolled by 64 rows so both SBUF
                    # inputs share a base partition (walrus constraint)
                    nc.vector.tensor_mul(
                        t2_t[0:64, :], ev[64:128, :], sin_sb[64:128, sl]
                    )
                    nc.vector.tensor_mul(
                        t2_t[64:128, :], ev[0:64, :], sin_sb[0:64, sl]
                    )
                    nc.vector.tensor_add(dest, t1_t[:], t2_t[:])

                for sg in range(SG):
                    hc = pah.tile([128, KT * 512], BF16, tag="hc")
                    h_view = hc[:].rearrange("p (a s) -> p a s", a=KT)
                    h_src = hT[:, sg * 512:(sg + 1) * 512].rearrange(
                        "(a p) s -> p a s", p=128
                    )
                    if sg == 0:
                        # interleave weight/hidden chunk loads so the PE can
                        # start as soon as the first k-tiles land
                        bounds = sorted(set([0, 1, 2, 4] + list(range(8, KT, CH)) + [KT]))
                        for lo, hi in zip(bounds[:-1], bounds[1:]):
                            csl = slice(lo, hi)
                            nc.sync.dma_start(w_view[:, csl, :], w_src[:, csl, :])
                            nc.sync.dma_start(h_view[:, csl, :], h_src[:, csl, :])
                            if lo == 28:
                                # needed only ~40us in; keep off the queue head
                                nc.sync.dma_start(eye_sb[:], eye[:])
                                nc.sync.dma_start(cos_sb[:], cosT[:])
                                nc.sync.dma_start(sin_sb[:], sinTe[:])
                                if mode == "causal":
                                    nc.sync.dma_start(
                                        m01_sb[:].rearrange("p (a i) -> p a i", a=4),
                                        m01[:].rearrange("(a p) i -> p a i", p=128),
                                    )
                    else:
                        nc.sync.dma_start(h_view[:], h_src[:])

                    pss = {
                        f: pap.tile([128, 512], F32, tag=f"proj{f}", name=f"proj{f}")
                        for f in ORDER
                    }
                    if sg < 2:
                        # k-major: tolerant of streaming DMA arrival
                        for k in range(KT):
                            for f in ORDER:
                                nc.tensor.matmul(
                                    pss[f][:],
                                    w_sb[:, k * FW + wcol(f):k * FW + wcol(f) + 128],
                                    hc[:, k * 512:(k + 1) * 512],
                                    start=(k == 0),
                                    stop=(k == KT - 1),
                                )
                        for f in ORDER:
                            evict_f(sg, f, pss[f])
                    else:
                        # f-major: evictions hide under the next group
                        for f in ORDER:
                            for k in range(KT):
                                nc.tensor.matmul(
                                    pss[f][:],
                                    w_sb[:, k * FW + wcol(f):k * FW + wcol(f) + 128],
                                    hc[:, k * 512:(k + 1) * 512],
                                    start=(k == 0),
                                    stop=(k == KT - 1),
                                )
                            evict_f(sg, f, pss[f])

            # ------------- Phase B+C: attention + output projection -------------
            with tc.tile_pool(name="late", bufs=1) as pl:
              # Wo transfer hides under early phase B compute
              wo_sb = pl.tile([128, QH * H], BF16, tag="wo")
              nc.sync.dma_start(
                  wo_sb[:].rearrange("p (a o) -> p a o", a=QH),
                  wo[:].rearrange("(a p) o -> p a o", p=128),
              )
              with (
                tc.tile_pool(name="phB_E", bufs=36) as pe_pool,
                tc.tile_pool(name="phB_acc", bufs=16) as pacc,
                tc.tile_pool(name="phB_tmp", bufs=3) as pbt,
                tc.tile_pool(name="phB_m", bufs=3) as pbm,
                tc.tile_pool(name="phB_ob", bufs=4) as pco,
                tc.tile_pool(name="phB_s", bufs=3, space="PSUM") as pbs,
                tc.tile_pool(name="phB_pv", bufs=2, space="PSUM") as pbv,
                tc.tile_pool(name="phB_den", bufs=1, space="PSUM") as pbd,
                tc.tile_pool(name="phB_po", bufs=2, space="PSUM") as pcp,
              ):
                  def emit_cchunk(st, ho):
                      """One (st, ho) block of the output projection:
                      out[st*128:(st+1)*128, ho*512:(ho+1)*512]."""
                      ssl = slice(st * 128, (st + 1) * 128)
                      po = pcp.tile([128, 512], F32, tag="po")
                      for f4 in range(QH):
                          nc.tensor.matmul(
                              po[:],
                              aT_sb[f4][:, ssl],
                              wo_sb[:, f4 * H + ho * 512:f4 * H + (ho + 1) * 512],
                              start=(f4 == 0),
                              stop=(f4 == QH - 1),
                          )
                      ob = pco.tile([128, 512], F16, tag="ob")
                      cp = nc.vector.tensor_copy if ho % 2 == 0 else nc.scalar.copy
                      cp(ob[:], po[:])
                      nc.sync.dma_start(out[ssl, ho * 512:(ho + 1) * 512], ob[:])

                  for ig in range(SG):
                      isl = slice(ig * 512, (ig + 1) * 512)
                      # causal: key tiles past this query block contribute
                      # exactly zero -- skip them entirely
                      jts = list(range(4 * ig + 4)) if mode == "causal" else list(range(ST))
                      n = len(jts)
                      for h in range(QH):
                          qh = qT_sb[h]
                          st_prev = 4 * (ig - 1) + h
                          chunks = list(range(H // 512)) if ig > 0 else []
                          ci = 0
                          et = []
                          pv = pbv.tile([128, 512], F32, tag="pv")

                          def emit_pv(idx):
                              jt = jts[idx]
                              nc.tensor.matmul(
                                  pv[:], v_sb[:, jt * HD:(jt + 1) * HD], et[idx][:],
                                  start=(idx == 0), stop=(idx == n - 1),
                              )

                          for idx, jt in enumerate(jts):
                              sps = pbs.tile([128, 512], F32, tag="s")
                              nc.tensor.matmul(
                                  sps[:],
                                  kT_sb[:, jt * 128:(jt + 1) * 128],
                                  qh[:, isl],
                                  start=True,
                                  stop=True,
                              )
                              e = pe_pool.tile([128, 512], BF16, tag="E", name=f"E{idx}")
                              et.append(e)
                              if masked:
                                  # host pre-scales maskT by sqrt(HD):
                                  # exp(SCALE*(scores + maskT)) == softmax logits
                                  mt = pbm.tile([128, 512], F32, tag="mT")
                                  nc.sync.dma_start(
                                      mt[:], maskT[jt * 128:(jt + 1) * 128, isl]
                                  )
                                  sm = pbm.tile([128, 512], F32, tag="sm")
                                  nc.vector.tensor_add(sm[:], sps[:], mt[:])
                                  nc.scalar.activation(
                                      e[:], sm[:],
                                      mybir.ActivationFunctionType.Exp,
                                      scale=SCALE,
                                  )
                              elif mode == "causal" and jt >= 4 * ig:
                                  # diagonal tile: exp then zero the j>i part
                                  p = jt - 4 * ig
                                  etmp = pbm.tile([128, 512], BF16, tag="etmp")
                                  nc.scalar.activation(
                                      etmp[:], sps[:],
                                      mybir.ActivationFunctionType.Exp,
                                      scale=SCALE,
                                  )
                                  nc.vector.tensor_mul(
                                      e[:], etmp[:],
                                      m01_sb[:, p * 512:(p + 1) * 512],
                                  )
                              else:
                                  nc.scalar.activation(
                                      e[:], sps[:],
                                      mybir.ActivationFunctionType.Exp,
                                      scale=SCALE,
                                  )
                              if idx >= 3:
                                  emit_pv(idx - 3)
                              if idx % 2 == 1 and ci < len(chunks):
                                  emit_cchunk(st_prev, chunks[ci])
                                  ci += 1
                          for idx in range(max(0, n - 3), n):
                              emit_pv(idx)
                          while ci < len(chunks):
                              emit_cchunk(st_prev, chunks[ci])
                              ci += 1
                          # denominator: bf16 add-tree over the E tiles, then a
                          # single ones-matmul reduces the last 128 partitions
                          cur = list(et)
                          lvl = 0
                          while len(cur) > 1:
                              nxt = []
                              for i in range(0, len(cur) - 1, 2):
                                  a = pacc.tile([128, 512], BF16, tag="acc", name=f"acc{lvl}_{i}")
                                  eng = nc.gpsimd if (lvl == 0 and i % 4 == 2) else nc.vector
                                  eng.tensor_add(a[:], cur[i][:], cur[i + 1][:])
                                  nxt.append(a)
                              if len(cur) % 2:
                                  nxt.append(cur[-1])
                              cur = nxt
                              lvl += 1
                          den = pbd.tile([128, 512], F32, tag="den")
                          nc.tensor.matmul(
                              den[:], ones_sb[:], cur[0][:], start=True, stop=True
                          )
                          rc = pbt.tile([128, 512], F32, tag="rc")
                          nc.vector.reciprocal_approx_fast(rc[:], den[:])
                          nc.vector.tensor_mul(aT_sb[h][:, isl], pv[:], rc[:])

                  # trailing output-projection blocks for the last query group
                  for h in range(QH):
                      st = 4 * (SG - 1) + h
                      for ho in range(H // 512):
                          emit_cchunk(st, ho)

    nc.finalize()
    return nc


def _get_kernel(mode: str):
    if mode not in _BUILT:
        _BUILT[mode] = _build(mode)
    return _BUILT[mode]


def _detect_mode(mask2d):
    if not np.any(mask2d):
        return "nomask"
    neg = mask2d[0, 1]
    if neg <= -1e4 and np.array_equal(
        mask2d, np.triu(np.full((S, S), neg, mask2d.dtype), k=1)
    ):
        return "causal"
    return "generic"


def kernel(hidden_states, position_ids, attention_mask, cos, sin, Wq, Wk, Wv, Wo,
           _collect_exec_info=None):
    hidden_states = np.asarray(hidden_states)
    attention_mask = np.asarray(attention_mask)
    cos = np.asarray(cos)
    sin = np.asarray(sin)
    Wq, Wk, Wv, Wo = (np.asarray(a) for a in (Wq, Wk, Wv, Wo))

    mode = _detect_mode(attention_mask[0, 0])
    masked = mode == "generic"
    nc = _get_kernel(mode)

    hT = np.ascontiguousarray(hidden_states[0].T).astype(NPBF16)
    cosT = np.ascontiguousarray(cos[0].T).astype(np.float32)
    sinTe = np.ascontiguousarray(sin[0].T).astype(np.float32)
    sinTe[:64] = -sinTe[:64]
    sinTe = np.ascontiguousarray(np.roll(sinTe, 64, axis=0))
    eye = np.eye(128, dtype=NPBF16)
    if mode == "causal":
        jj = np.arange(128)[:, None]
        ii = np.arange(512)[None, :]
        m01 = np.concatenate(
            [(128 * p + jj <= ii).astype(NPBF16) for p in range(4)], axis=0
        )

    in_maps = []
    for c in range(N_CORES):
        wqkv = np.concatenate(
            [
                Wq[:, c * F:(c + 1) * F],
                Wk[:, c * HD:(c + 1) * HD],
                Wv[:, c * HD:(c + 1) * HD],
            ],
            axis=1,
        ).astype(NPBF16)
        m = {
            "hT": hT,
            "wqkv": wqkv,
            "wo": Wo[c * F:(c + 1) * F, :].astype(NPBF16),
            "cosT": cosT,
            "sinTe": sinTe,
            "eye": eye,
        }
        if masked:
            m["maskT"] = (
                np.ascontiguousarray(attention_mask[0, 0].T).astype(np.float32)
                * math.sqrt(HD)
            )
        if mode == "causal":
            m["m01"] = m01
        in_maps.append(m)

    trace = _collect_exec_info is not None
    res = run_bass_kernel_spmd(nc, in_maps, list(range(N_CORES)), trace=trace)
    if trace:
        _collect_exec_info["exec_time_ns"] = res.exec_time_ns
        _collect_exec_info["results"] = res

    acc = res.results[0]["out"].astype(np.float64)
    for c in range(1, N_CORES):
        acc += res.results[c]["out"].astype(np.float64)
    return acc.astype(np.float32)[None, :, :]


# revision 7
# speedup vs baseline: 1.1302x; 1.0287x over previous
"""Multi-head attention (GQA, 32 q-heads / 8 kv-heads, S=2048, H=4096) on 8
Trainium2 NeuronCores.

Sharding: tensor-parallel across heads. Core c owns kv-head c and q-heads
4c..4c+3 (Wq/Wk/Wv column-sharded, Wo row-sharded). Each core computes a
partial output [S, H] in fp16; the host sums the 8 partials.

Per-core dataflow (everything bf16 into the PE, fp32 accumulation):
  A) qT/kT/vT = W.T @ hiddenT  (weights stationary, hiddenT moving)
     + RoPE applied in the transposed [hd, s] layout
     + vT transposed back to natural v[s, hd] via PE-transpose
     sg0/sg1 run k-major (DMA-paced); sg2/sg3 run f-major so PSUM
     evictions hide under the next feature group's matmuls.
  B) per (q-group ig, head h): scoresT[j,i] = kT.T @ qT -> E = exp(...)
     denom via a bf16 add-tree over the 16 E tiles (DVE+Pool) and a
     single ones-matmul; attnT[d,i] = v.T @ E, normalized on DVE.
  C) partial_out[s,:] = attnT.T @ Wo_c, interleaved INTO phase B: the
     C-block for query-group ig-1 is issued between the scores matmuls
     of group ig so the PE stays fed while ACT drains the exps.
"""

import math
import os
import sys

if os.path.isdir("/opt/trn_rl_repo") and "/opt/trn_rl_repo" not in sys.path:
    sys.path.insert(0, "/opt/trn_rl_repo")

import numpy as np
import ml_dtypes

import concourse.bacc as bacc
import concourse.mybir as mybir
from concourse import tile
from concourse.bass_utils import run_bass_kernel_spmd

BF16 = mybir.dt.bfloat16
F16 = mybir.dt.float16
F32 = mybir.dt.float32
NPBF16 = ml_dtypes.bfloat16

S = 2048
H = 4096
HD = 128
NH = 32
NKV = 8
N_CORES = 8
QH = NH // N_CORES          # q-heads per core = 4
F = QH * HD                 # q feature columns per core = 512
KT = H // 128               # contraction tiles for the projections = 32
ST = S // 128               # 128-row tiles along S = 16
SG = S // 512               # 512-wide groups along S = 4
SCALE = 1.0 / math.sqrt(HD)

_BUILT = {}

# feature-group order: k first (phase B needs kT complete first), then v,
# then the q heads
ORDER = (4, 5, 0, 1, 2, 3)


def _build(mode: str):
    masked = mode == "generic"
    nc = bacc.Bacc(None, target_bir_lowering=False)

    hT = nc.declare_dram_parameter("hT", [H, S], BF16, isOutput=False)
    wqkv = nc.declare_dram_parameter("wqkv", [H, F + 2 * HD], BF16, isOutput=False)
    wo = nc.declare_dram_parameter("wo", [F, H], BF16, isOutput=False)
    cosT = nc.declare_dram_parameter("cosT", [HD, S], F32, isOutput=False)
    sinTe = nc.declare_dram_parameter("sinTe", [HD, S], F32, isOutput=False)
    eye = nc.declare_dram_parameter("eye", [128, 128], BF16, isOutput=False)
    if masked:
        maskT = nc.declare_dram_parameter("maskT", [S, S], F32, isOutput=False)
    if mode == "causal":
        # four 0/1 diagonal-tile patterns, stacked [4*128, 512]
        m01 = nc.declare_dram_parameter("m01", [4 * 128, 512], BF16, isOutput=False)
    out = nc.declare_dram_parameter("out", [S, H], F16, isOutput=True)

    FW = F + 2 * HD  # 768 weight columns per contraction tile

    with tile.TileContext(nc) as tc:
        with tc.tile_pool(name="persist", bufs=1) as pp:
            # persistent SBUF tensors
            cos_sb = pp.tile([HD, S], F32, tag="cos")
            sin_sb = pp.tile([HD, S], F32, tag="sin")
            eye_sb = pp.tile([128, 128], BF16, tag="eye")
            ones_sb = pp.tile([128, 128], BF16, tag="ones")
            qT_sb = [pp.tile([HD, S], BF16, tag=f"qT{h}", name=f"qT{h}") for h in range(QH)]
            kT_sb = pp.tile([HD, S], BF16, tag="kT")
            v_sb = pp.tile([128, ST * HD], BF16, tag="v")  # block jt: v[jt*128:(jt+1)*128, :]
            aT_sb = [pp.tile([HD, S], BF16, tag=f"aT{h}", name=f"aT{h}") for h in range(QH)]
            # RoPE scratch lives in the persistent pool so phase-B tiles
            # never alias it
            vt_t = pp.tile([128, 512], BF16, tag="vt")
            ev_t = {
                f: pp.tile([128, 512], F32, tag=f"ev{f}", name=f"ev{f}")
                for f in (4, 0, 1, 2, 3)
            }
            t1_t = pp.tile([128, 512], F32, tag="t1")
            t2_t = pp.tile([128, 512], F32, tag="t2")
            if mode == "causal":
                m01_sb = pp.tile([128, 4 * 512], BF16, tag="m01")

            nc.gpsimd.memset(ones_sb[:], 1.0)

            # wqkv column ranges per feature group
            def wcol(f):
                base = f * 128 if f < QH else F + (f - QH) * HD
                return base

            # ---------------- Phase A: projections + RoPE ----------------
            with (
                tc.tile_pool(name="phA_sb", bufs=1) as pa,
                tc.tile_pool(name="phA_h", bufs=2) as pah,
                tc.tile_pool(name="phA_ps", bufs=1, space="PSUM") as pap,
                tc.tile_pool(name="phA_pst", bufs=2, space="PSUM") as papt,
            ):
                w_sb = pa.tile([128, KT * FW], BF16, tag="wqkv")
                w_view = w_sb[:].rearrange("p (a f) -> p a f", a=KT)
                w_src = wqkv[:].rearrange("(a p) f -> p a f", p=128)
                CH = 4  # ktiles per DMA chunk

                pending_tr = []

                def flush_tr():
                    for fn in pending_tr:
                        fn()
                    pending_tr.clear()

                def evict_f(sg, f, pss_f):
                    """PSUM -> SBUF eviction + RoPE (or v transpose) for one
                    feature group."""
                    sl = slice(sg * 512, (sg + 1) * 512)
                    if f == 5:
                        # v: copy out now; the PE transposes back to natural
                        # [s, hd] layout are deferred until after the next
                        # matmul group so the PE never waits on the copy
                        nc.vector.tensor_copy(vt_t[:], pss_f[:])

                        def tr(sg=sg):
                            for b in range(4):
                                jt = sg * 4 + b
                                pst = papt.tile([128, 128], BF16, tag="vtr")
                                nc.tensor.transpose(
                                    pst[:], vt_t[:, b * 128:(b + 1) * 128], eye_sb[:]
                                )
                                cp = nc.vector.tensor_copy if b % 2 == 0 else nc.scalar.copy
                                cp(v_sb[:, jt * HD:(jt + 1) * HD], pst[:])

                        pending_tr.append(tr)
                        return
                    ev = ev_t[f]
                    if f in (0, 2):
                        nc.vector.tensor_copy(ev[:], pss_f[:])
                    else:
                        nc.scalar.copy(ev[:], pss_f[:])
                    # RoPE: out[d] = x[d]*cos[d] + x[(d+64)%128]*sinTe[d]
                    dest = (qT_sb[f] if f < QH else kT_sb)[:, sl]
                    nc.vector.tensor_mul(t1_t[:], ev[:], cos_sb[:, sl])
                    # sin table is host-rolled by 64 rows so both SBUF
                    # inputs share a base partition (walrus constraint)
                    nc.vector.tensor_mul(
                        t2_t[0:64, :], ev[64:128, :], sin_sb[64:128, sl]
                    )
                    nc.vector.tensor_mul(
                        t2_t[64:128, :], ev[0:64, :], sin_sb[0:64, sl]
                    )
                    nc.vector.tensor_add(dest, t1_t[:], t2_t[:])

                for sg in range(SG):
                    hc = pah.tile([128, KT * 512], BF16, tag="hc")
                    h_view = hc[:].rearrange("p (a s) -> p a s", a=KT)
                    h_src = hT[:, sg * 512:(sg + 1) * 512].rearrange(
                        "(a p) s -> p a s", p=128
                    )
                    if sg == 0:
                        # weights stream on the sync queue, hidden chunks on
                        # the scalar queue: the two transfers run in parallel
                        # and the PE can start as soon as k-tile 0 lands.
                        # First weight piece is just the kv columns of k-tile
                        # 0 -- the very first matmul group (f=4) needs only
                        # those.
                        nc.sync.dma_start(w_view[:, 0:1, F:FW], w_src[:, 0:1, F:FW])
                        nc.scalar.dma_start(h_view[:, 0:1, :], h_src[:, 0:1, :])
                        nc.sync.dma_start(w_view[:, 0:1, 0:F], w_src[:, 0:1, 0:F])
                        bounds = sorted(set([1, 2, 3, 4, 6] + list(range(8, KT, CH)) + [KT]))
                        for lo, hi in zip(bounds[:-1], bounds[1:]):
                            csl = slice(lo, hi)
                            nc.sync.dma_start(w_view[:, csl, :], w_src[:, csl, :])
                            nc.scalar.dma_start(h_view[:, csl, :], h_src[:, csl, :])
                            if lo == 28:
                                # needed only ~40us in; keep off the queue head
                                nc.sync.dma_start(eye_sb[:], eye[:])
                                nc.sync.dma_start(cos_sb[:], cosT[:])
                                nc.sync.dma_start(sin_sb[:], sinTe[:])
                                if mode == "causal":
                                    nc.sync.dma_start(
                                        m01_sb[:].rearrange("p (a i) -> p a i", a=4),
                                        m01[:].rearrange("(a p) i -> p a i", p=128),
                                    )
                    else:
                        nc.scalar.dma_start(h_view[:], h_src[:])

                    pss = {
                        f: pap.tile([128, 512], F32, tag=f"proj{f}", name=f"proj{f}")
                        for f in ORDER
                    }
                    if sg < 1:
                        # k-major: tolerant of streaming DMA arrival
                        for k in range(KT):
                            for f in ORDER:
                                nc.tensor.matmul(
                                    pss[f][:],
                                    w_sb[:, k * FW + wcol(f):k * FW + wcol(f) + 128],
                                    hc[:, k * 512:(k + 1) * 512],
                                    start=(k == 0),
                                    stop=(k == KT - 1),
                                )
                        for f in ORDER:
                            evict_f(sg, f, pss[f])
                    else:
                        # f-major: evictions hide under the next group
                        for f in ORDER:
                            for k in range(KT):
                                nc.tensor.matmul(
                                    pss[f][:],
                                    w_sb[:, k * FW + wcol(f):k * FW + wcol(f) + 128],
                                    hc[:, k * 512:(k + 1) * 512],
                                    start=(k == 0),
                                    stop=(k == KT - 1),
                                )
                            flush_tr()
                            evict_f(sg, f, pss[f])
                flush_tr()

            # ------------- Phase B+C: attention + output projection -------------
            with tc.tile_pool(name="late", bufs=1) as pl:
              # Wo transfer hides under early phase B compute
              wo_sb = pl.tile([128, QH * H], BF16, tag="wo")
              nc.sync.dma_start(
                  wo_sb[:].rearrange("p (a o) -> p a o", a=QH),
                  wo[:].rearrange("(a p) o -> p a o", p=128),
              )
              with (
                tc.tile_pool(name="phB_E", bufs=18) as pe_pool,
                tc.tile_pool(name="phB_acc", bufs=16) as pacc,
                tc.tile_pool(name="phB_tmp", bufs=3) as pbt,
                tc.tile_pool(name="phB_m", bufs=3) as pbm,
                tc.tile_pool(name="phB_ob", bufs=4) as pco,
                tc.tile_pool(name="phB_s", bufs=2, space="PSUM") as pbs,
                tc.tile_pool(name="phB_pv", bufs=1, space="PSUM") as pbv,
                tc.tile_pool(name="phB_den", bufs=1, space="PSUM") as pbd,
                tc.tile_pool(name="phB_po", bufs=2, space="PSUM") as pcp,
              ):
                  def emit_cchunk(st, ho):
                      """One (st, ho) block of the output projection:
                      out[st*128:(st+1)*128, ho*512:(ho+1)*512]."""
                      ssl = slice(st * 128, (st + 1) * 128)
                      po = pcp.tile([128, 512], F32, tag="po")
                      for f4 in range(QH):
                          nc.tensor.matmul(
                              po[:],
                              aT_sb[f4][:, ssl],
                              wo_sb[:, f4 * H + ho * 512:f4 * H + (ho + 1) * 512],
                              start=(f4 == 0),
                              stop=(f4 == QH - 1),
                          )
                      ob = pco.tile([128, 512], F16, tag="ob")
                      cp = nc.vector.tensor_copy if ho % 2 == 0 else nc.scalar.copy
                      cp(ob[:], po[:])
                      nc.sync.dma_start(out[ssl, ho * 512:(ho + 1) * 512], ob[:])

                  for ig in range(SG):
                      isl = slice(ig * 512, (ig + 1) * 512)
                      # causal: key tiles past this query block contribute
                      # exactly zero -- skip them entirely
                      jts = list(range(4 * ig + 4)) if mode == "causal" else list(range(ST))
                      n = len(jts)
                      npair = n // 2
                      for h in range(QH):
                          qh = qT_sb[h]
                          st_prev = 4 * (ig - 1) + h
                          chunks = list(range(H // 512)) if ig > 0 else []
                          ci = 0
                          et = []
                          pv = pbv.tile([128, 512], F32, tag="pv")
                          T = None  # running denominator partial sum

                          def emit_pv(idx):
                              jt = jts[idx]
                              src = et[idx // 2][:, (idx % 2) * 512:(idx % 2 + 1) * 512]
                              nc.tensor.matmul(
                                  pv[:], v_sb[:, jt * HD:(jt + 1) * HD], src,
                                  start=(idx == 0), stop=(idx == n - 1),
                              )

                          for p in range(npair):
                              sps = pbs.tile([128, 1024], F32, tag="s")
                              diag = False
                              for half in range(2):
                                  idx = 2 * p + half
                                  jt = jts[idx]
                                  nc.tensor.matmul(
                                      sps[:, half * 512:(half + 1) * 512],
                                      kT_sb[:, jt * 128:(jt + 1) * 128],
                                      qh[:, isl],
                                      start=True,
                                      stop=True,
                                  )
                                  if mode == "causal" and jt >= 4 * ig:
                                      diag = True
                                  if idx >= 4:
                                      emit_pv(idx - 4)
                                  if half == 1 and ci < len(chunks):
                                      emit_cchunk(st_prev, chunks[ci])
                                      ci += 1
                              e = pe_pool.tile([128, 1024], BF16, tag="E", name=f"E{p}")
                              et.append(e)
                              if masked:
                                  # host pre-scales maskT by sqrt(HD):
                                  # exp(SCALE*(scores + maskT)) == softmax logits
                                  for half in range(2):
                                      jt = jts[2 * p + half]
                                      hsl = slice(half * 512, (half + 1) * 512)
                                      mt = pbm.tile([128, 512], F32, tag="mT")
                                      nc.sync.dma_start(
                                          mt[:], maskT[jt * 128:(jt + 1) * 128, isl]
                                      )
                                      sm = pbm.tile([128, 512], F32, tag="sm")
                                      nc.vector.tensor_add(sm[:], sps[:, hsl], mt[:])
                                      nc.scalar.activation(
                                          e[:, hsl], sm[:],
                                          mybir.ActivationFunctionType.Exp,
                                          scale=SCALE,
                                      )
                              elif diag:
                                  # pair contains diagonal tiles: exp then zero
                                  # the j>i part per half
                                  for half in range(2):
                                      jt = jts[2 * p + half]
                                      hsl = slice(half * 512, (half + 1) * 512)
                                      if jt >= 4 * ig:
                                          dp = jt - 4 * ig
                                          etmp = pbm.tile([128, 512], BF16, tag="etmp")
                                          nc.scalar.activation(
                                              etmp[:], sps[:, hsl],
                                              mybir.ActivationFunctionType.Exp,
                                              scale=SCALE,
                                          )
                                          nc.vector.tensor_mul(
                                              e[:, hsl], etmp[:],
                                              m01_sb[:, dp * 512:(dp + 1) * 512],
                                          )
                                      else:
                                          nc.scalar.activation(
                                              e[:, hsl], sps[:, hsl],
                                              mybir.ActivationFunctionType.Exp,
                                              scale=SCALE,
                                          )
                              else:
                                  # fused: one activation covers both halves
                                  nc.scalar.activation(
                                      e[:], sps[:],
                                      mybir.ActivationFunctionType.Exp,
                                      scale=SCALE,
                                  )
                              # denominator: pair-sum then running-sum chain,
                              # emitted inline so the adds run as exps land.
                              # The last pair stays off the Pool engine (its
                              # ~1.2us add would sit on the critical tail).
                              a = pacc.tile([128, 512], BF16, tag="acc", name=f"A{p}")
                              eng = nc.gpsimd if (p in (1, 3, 5) and p != npair - 1) else nc.vector
                              eng.tensor_add(a[:], e[:, 0:512], e[:, 512:1024])
                              if T is None:
                                  T = a
                              else:
                                  Tn = pacc.tile([128, 512], BF16, tag="acc", name=f"T{p}")
                                  nc.vector.tensor_add(Tn[:], T[:], a[:])
                                  T = Tn
                          while ci < len(chunks):
                              emit_cchunk(st_prev, chunks[ci])
                              ci += 1
                          for idx in range(max(0, n - 4), n):
                              emit_pv(idx)
                          den = pbd.tile([128, 512], F32, tag="den")
                          nc.tensor.matmul(
                              den[:], ones_sb[:], T[:], start=True, stop=True
                          )
                          rc = pbt.tile([128, 512], F32, tag="rc")
                          nc.vector.reciprocal_approx_fast(rc[:], den[:])
                          nc.vector.tensor_mul(aT_sb[h][:, isl], pv[:], rc[:])

                  # trailing output-projection blocks for the last query group
                  for h in range(QH):
                      st = 4 * (SG - 1) + h
                      for ho in range(H // 512):
                          emit_cchunk(st, ho)

    nc.finalize()
    return nc


def _get_kernel(mode: str):
    if mode not in _BUILT:
        _BUILT[mode] = _build(mode)
    return _BUILT[mode]


def _detect_mode(mask2d):
    if not np.any(mask2d):
        return "nomask"
    neg = mask2d[0, 1]
    if neg <= -1e4 and np.array_equal(
        mask2d, np.triu(np.full((S, S), neg, mask2d.dtype), k=1)
    ):
        return "causal"
    return "generic"


def kernel(hidden_states, position_ids, attention_mask, cos, sin, Wq, Wk, Wv, Wo,
           _collect_exec_info=None):
    hidden_states = np.asarray(hidden_states)
    attention_mask = np.asarray(attention_mask)
    cos = np.asarray(cos)
    sin = np.asarray(sin)
    Wq, Wk, Wv, Wo = (np.asarray(a) for a in (Wq, Wk, Wv, Wo))

    mode = _detect_mode(attention_mask[0, 0])
    masked = mode == "generic"
    nc = _get_kernel(mode)

    hT = np.ascontiguousarray(hidden_states[0].T).astype(NPBF16)
    cosT = np.ascontiguousarray(cos[0].T).astype(np.float32)
    sinTe = np.ascontiguousarray(sin[0].T).astype(np.float32)
    sinTe[:64] = -sinTe[:64]
    sinTe = np.ascontiguousarray(np.roll(sinTe, 64, axis=0))
    eye = np.eye(128, dtype=NPBF16)
    if mode == "causal":
        jj = np.arange(128)[:, None]
        ii = np.arange(512)[None, :]
        m01 = np.concatenate(
            [(128 * p + jj <= ii).astype(NPBF16) for p in range(4)], axis=0
        )

    in_maps = []
    for c in range(N_CORES):
        wqkv = np.concatenate(
            [
                Wq[:, c * F:(c + 1) * F],
                Wk[:, c * HD:(c + 1) * HD],
                Wv[:, c * HD:(c + 1) * HD],
            ],
            axis=1,
        ).astype(NPBF16)
        m = {
            "hT": hT,
            "wqkv": wqkv,
            "wo": Wo[c * F:(c + 1) * F, :].astype(NPBF16),
            "cosT": cosT,
            "sinTe": sinTe,
            "eye": eye,
        }
        if masked:
            m["maskT"] = (
                np.ascontiguousarray(attention_mask[0, 0].T).astype(np.float32)
                * math.sqrt(HD)
            )
        if mode == "causal":
            m["m01"] = m01
        in_maps.append(m)

    trace = _collect_exec_info is not None
    res = run_bass_kernel_spmd(nc, in_maps, list(range(N_CORES)), trace=trace)
    if trace:
        _collect_exec_info["exec_time_ns"] = res.exec_time_ns
        _collect_exec_info["results"] = res

    acc = res.results[0]["out"].astype(np.float64)
    for c in range(1, N_CORES):
        acc += res.results[c]["out"].astype(np.float64)
    return acc.astype(np.float32)[None, :, :]


# revision 9
# speedup vs baseline: 1.1486x; 1.0162x over previous
"""Multi-head attention (GQA, 32 q-heads / 8 kv-heads, S=2048, H=4096) on 8
Trainium2 NeuronCores.

Sharding: tensor-parallel across heads. Core c owns kv-head c and q-heads
4c..4c+3 (Wq/Wk/Wv column-sharded, Wo row-sharded). Each core computes a
partial output [S, H] in fp16; the host sums the 8 partials.

Per-core dataflow (everything bf16 into the PE, fp32 accumulation):
  A) qT/kT/vT = W.T @ hiddenT  (weights stationary, hiddenT moving)
     + RoPE applied in the transposed [hd, s] layout
     + vT transposed back to natural v[s, hd] via PE-transpose
     sg0/sg1 run k-major (DMA-paced); sg2/sg3 run f-major so PSUM
     evictions hide under the next feature group's matmuls.
  B) per (q-group ig, head h): scoresT[j,i] = kT.T @ qT -> E = exp(...)
     denom via a bf16 add-tree over the 16 E tiles (DVE+Pool) and a
     single ones-matmul; attnT[d,i] = v.T @ E, normalized on DVE.
  C) partial_out[s,:] = attnT.T @ Wo_c, interleaved INTO phase B: the
     C-block for query-group ig-1 is issued between the scores matmuls
     of group ig so the PE stays fed while ACT drains the exps.
"""

import math
import os
import sys

if os.path.isdir("/opt/trn_rl_repo") and "/opt/trn_rl_repo" not in sys.path:
    sys.path.insert(0, "/opt/trn_rl_repo")

import numpy as np
import ml_dtypes

import concourse.bacc as bacc
import concourse.mybir as mybir
from concourse import tile
from concourse.bass_utils import run_bass_kernel_spmd

BF16 = mybir.dt.bfloat16
F16 = mybir.dt.float16
F32 = mybir.dt.float32
NPBF16 = ml_dtypes.bfloat16

S = 2048
H = 4096
HD = 128
NH = 32
NKV = 8
N_CORES = 8
QH = NH // N_CORES          # q-heads per core = 4
F = QH * HD                 # q feature columns per core = 512
KT = H // 128               # contraction tiles for the projections = 32
ST = S // 128               # 128-row tiles along S = 16
SG = S // 512               # 512-wide groups along S = 4
SCALE = 1.0 / math.sqrt(HD)

_BUILT = {}

# feature-group order: k first (phase B needs kT complete first), then v,
# then the q heads
ORDER = (4, 5, 0, 1, 2, 3)


def _build(mode: str):
    masked = mode == "generic"
    nc = bacc.Bacc(None, target_bir_lowering=False)

    hT = nc.declare_dram_parameter("hT", [H, S], BF16, isOutput=False)
    wqkv = nc.declare_dram_parameter("wqkv", [H, F + 2 * HD], BF16, isOutput=False)
    wo = nc.declare_dram_parameter("wo", [F, H], BF16, isOutput=False)
    cosT = nc.declare_dram_parameter("cosT", [HD, S], F32, isOutput=False)
    sinTe = nc.declare_dram_parameter("sinTe", [HD, S], F32, isOutput=False)
    eye = nc.declare_dram_parameter("eye", [128, 128], BF16, isOutput=False)
    if masked:
        maskT = nc.declare_dram_parameter("maskT", [S, S], F32, isOutput=False)
    if mode == "causal":
        # four 0/1 diagonal-tile patterns, stacked [4*128, 512]
        m01 = nc.declare_dram_parameter("m01", [4 * 128, 512], BF16, isOutput=False)
    out = nc.declare_dram_parameter("out", [S, H], F16, isOutput=True)

    FW = F + 2 * HD  # 768 weight columns per contraction tile

    with tile.TileContext(nc) as tc:
        with tc.tile_pool(name="persist", bufs=1) as pp:
            # persistent SBUF tensors
            cos_sb = pp.tile([HD, S], F32, tag="cos")
            sin_sb = pp.tile([HD, S], F32, tag="sin")
            eye_sb = pp.tile([128, 128], BF16, tag="eye")
            ones_sb = pp.tile([128, 128], BF16, tag="ones")
            qT_sb = [pp.tile([HD, S], BF16, tag=f"qT{h}", name=f"qT{h}") for h in range(QH)]
            kT_sb = pp.tile([HD, S], BF16, tag="kT")
            v_sb = pp.tile([128, ST * HD], BF16, tag="v")  # block jt: v[jt*128:(jt+1)*128, :]
            aT_sb = [pp.tile([HD, S], BF16, tag=f"aT{h}", name=f"aT{h}") for h in range(QH)]
            # RoPE scratch lives in the persistent pool so phase-B tiles
            # never alias it
            vt_t = pp.tile([128, 512], BF16, tag="vt")
            ev_t = {
                f: pp.tile([128, 512], F32, tag=f"ev{f}", name=f"ev{f}")
                for f in (4, 0, 1, 2, 3)
            }
            t1_t = pp.tile([128, 512], F32, tag="t1")
            t2_t = pp.tile([128, 512], F32, tag="t2")
            if mode == "causal":
                m01_sb = pp.tile([128, 4 * 512], BF16, tag="m01")

            nc.gpsimd.memset(ones_sb[:], 1.0)

            # wqkv column ranges per feature group
            def wcol(f):
                base = f * 128 if f < QH else F + (f - QH) * HD
                return base

            # ---------------- Phase A: projections + RoPE ----------------
            with (
                tc.tile_pool(name="phA_sb", bufs=1) as pa,
                tc.tile_pool(name="phA_h", bufs=2) as pah,
                tc.tile_pool(name="phA_ps", bufs=1, space="PSUM") as pap,
                tc.tile_pool(name="phA_pst", bufs=2, space="PSUM") as papt,
            ):
                w_sb = pa.tile([128, KT * FW], BF16, tag="wqkv")
                w_view = w_sb[:].rearrange("p (a f) -> p a f", a=KT)
                w_src = wqkv[:].rearrange("(a p) f -> p a f", p=128)
                CH = 4  # ktiles per DMA chunk

                pending_tr = []

                def flush_tr():
                    for fn in pending_tr:
                        fn()
                    pending_tr.clear()

                def evict_f(sg, f, pss_f):
                    """PSUM -> SBUF eviction + RoPE (or v transpose) for one
                    feature group."""
                    sl = slice(sg * 512, (sg + 1) * 512)
                    if f == 5:
                        # v: copy out now; the PE transposes back to natural
                        # [s, hd] layout are deferred until after the next
                        # matmul group so the PE never waits on the copy
                        nc.vector.tensor_copy(vt_t[:], pss_f[:])

                        def tr(sg=sg):
                            for b in range(4):
                                jt = sg * 4 + b
                                pst = papt.tile([128, 128], BF16, tag="vtr")
                                nc.tensor.transpose(
                                    pst[:], vt_t[:, b * 128:(b + 1) * 128], eye_sb[:]
                                )
                                cp = nc.vector.tensor_copy if b % 2 == 0 else nc.scalar.copy
                                cp(v_sb[:, jt * HD:(jt + 1) * HD], pst[:])

                        pending_tr.append(tr)
                        return
                    ev = ev_t[f]
                    if f in (0, 2):
                        nc.vector.tensor_copy(ev[:], pss_f[:])
                    else:
                        nc.scalar.copy(ev[:], pss_f[:])
                    # RoPE: out[d] = x[d]*cos[d] + x[(d+64)%128]*sinTe[d]
                    dest = (qT_sb[f] if f < QH else kT_sb)[:, sl]
                    nc.vector.tensor_mul(t1_t[:], ev[:], cos_sb[:, sl])
                    # sin table is host-rolled by 64 rows so both SBUF
                    # inputs share a base partition (walrus constraint)
                    nc.vector.tensor_mul(
                        t2_t[0:64, :], ev[64:128, :], sin_sb[64:128, sl]
                    )
                    nc.vector.tensor_mul(
                        t2_t[64:128, :], ev[0:64, :], sin_sb[0:64, sl]
                    )
                    nc.vector.tensor_add(dest, t1_t[:], t2_t[:])

                for sg in range(SG):
                    hc = pah.tile([128, KT * 512], BF16, tag="hc")
                    h_view = hc[:].rearrange("p (a s) -> p a s", a=KT)
                    h_src = hT[:, sg * 512:(sg + 1) * 512].rearrange(
                        "(a p) s -> p a s", p=128
                    )
                    if sg == 0:
                        # weights stream on the sync queue, hidden chunks on
                        # the scalar queue: the two transfers run in parallel
                        # and the PE can start as soon as k-tile 0 lands.
                        # First weight piece is just the kv columns of k-tile
                        # 0 -- the very first matmul group (f=4) needs only
                        # those.
                        nc.sync.dma_start(w_view[:, 0:1, F:FW], w_src[:, 0:1, F:FW])
                        nc.scalar.dma_start(h_view[:, 0:1, :], h_src[:, 0:1, :])
                        nc.sync.dma_start(w_view[:, 0:1, 0:F], w_src[:, 0:1, 0:F])
                        bounds = sorted(set([1, 2, 3, 4, 6] + list(range(8, KT, CH)) + [KT]))
                        for lo, hi in zip(bounds[:-1], bounds[1:]):
                            csl = slice(lo, hi)
                            nc.sync.dma_start(w_view[:, csl, :], w_src[:, csl, :])
                            nc.scalar.dma_start(h_view[:, csl, :], h_src[:, csl, :])
                            if lo == 28:
                                # needed only ~40us in; keep off the queue head
                                nc.sync.dma_start(eye_sb[:], eye[:])
                                nc.sync.dma_start(cos_sb[:], cosT[:])
                                nc.sync.dma_start(sin_sb[:], sinTe[:])
                                if mode == "causal":
                                    nc.sync.dma_start(
                                        m01_sb[:].rearrange("p (a i) -> p a i", a=4),
                                        m01[:].rearrange("(a p) i -> p a i", p=128),
                                    )
                    else:
                        # chunked so early k-tiles unblock as they land (a
                        # single dma_start only signals at full completion)
                        for lo in range(0, KT, 8):
                            nc.scalar.dma_start(
                                h_view[:, lo:lo + 8, :], h_src[:, lo:lo + 8, :]
                            )

                    pss = {
                        f: pap.tile([128, 512], F32, tag=f"proj{f}", name=f"proj{f}")
                        for f in ORDER
                    }
                    if sg < 2:
                        # k-major: tolerant of streaming DMA arrival
                        for k in range(KT):
                            for f in ORDER:
                                nc.tensor.matmul(
                                    pss[f][:],
                                    w_sb[:, k * FW + wcol(f):k * FW + wcol(f) + 128],
                                    hc[:, k * 512:(k + 1) * 512],
                                    start=(k == 0),
                                    stop=(k == KT - 1),
                                )
                        for f in ORDER:
                            # flush the previous sg's deferred v-transposes
                            # before this sg's v eviction reuses vt_t
                            if f == 5:
                                flush_tr()
                            evict_f(sg, f, pss[f])
                    else:
                        # f-major: evictions hide under the next group
                        for f in ORDER:
                            for k in range(KT):
                                nc.tensor.matmul(
                                    pss[f][:],
                                    w_sb[:, k * FW + wcol(f):k * FW + wcol(f) + 128],
                                    hc[:, k * 512:(k + 1) * 512],
                                    start=(k == 0),
                                    stop=(k == KT - 1),
                                )
                            flush_tr()
                            evict_f(sg, f, pss[f])
                flush_tr()

            # ------------- Phase B+C: attention + output projection -------------
            with tc.tile_pool(name="late", bufs=1) as pl:
              # Wo transfer hides under early phase B compute
              wo_sb = pl.tile([128, QH * H], BF16, tag="wo")
              nc.sync.dma_start(
                  wo_sb[:].rearrange("p (a o) -> p a o", a=QH),
                  wo[:].rearrange("(a p) o -> p a o", p=128),
              )
              with (
                tc.tile_pool(name="phB_E", bufs=18) as pe_pool,
                tc.tile_pool(name="phB_acc", bufs=16) as pacc,
                tc.tile_pool(name="phB_tmp", bufs=3) as pbt,
                tc.tile_pool(name="phB_m", bufs=3) as pbm,
                tc.tile_pool(name="phB_ob", bufs=4) as pco,
                tc.tile_pool(name="phB_s", bufs=2, space="PSUM") as pbs,
                tc.tile_pool(name="phB_pv", bufs=1, space="PSUM") as pbv,
                tc.tile_pool(name="phB_den", bufs=1, space="PSUM") as pbd,
                tc.tile_pool(name="phB_po", bufs=2, space="PSUM") as pcp,
              ):
                  def emit_cchunk(st, ho):
                      """One (st, ho) block of the output projection:
                      out[st*128:(st+1)*128, ho*512:(ho+1)*512]."""
                      ssl = slice(st * 128, (st + 1) * 128)
                      po = pcp.tile([128, 512], F32, tag="po")
                      for f4 in range(QH):
                          nc.tensor.matmul(
                              po[:],
                              aT_sb[f4][:, ssl],
                              wo_sb[:, f4 * H + ho * 512:f4 * H + (ho + 1) * 512],
                              start=(f4 == 0),
                              stop=(f4 == QH - 1),
                          )
                      ob = pco.tile([128, 512], F16, tag="ob")
                      cp = nc.vector.tensor_copy if ho % 2 == 0 else nc.scalar.copy
                      cp(ob[:], po[:])
                      nc.sync.dma_start(out[ssl, ho * 512:(ho + 1) * 512], ob[:])

                  for ig in range(SG):
                      isl = slice(ig * 512, (ig + 1) * 512)
                      # causal: key tiles past this query block contribute
                      # exactly zero -- skip them entirely
                      jts = list(range(4 * ig + 4)) if mode == "causal" else list(range(ST))
                      n = len(jts)
                      npair = n // 2
                      for h in range(QH):
                          qh = qT_sb[h]
                          st_prev = 4 * (ig - 1) + h
                          chunks = list(range(H // 512)) if ig > 0 else []
                          ci = 0
                          et = []
                          pv = pbv.tile([128, 512], F32, tag="pv")
                          T = None  # running denominator partial sum

                          def emit_pv(idx):
                              jt = jts[idx]
                              src = et[idx // 2][:, (idx % 2) * 512:(idx % 2 + 1) * 512]
                              nc.tensor.matmul(
                                  pv[:], v_sb[:, jt * HD:(jt + 1) * HD], src,
                                  start=(idx == 0), stop=(idx == n - 1),
                              )

                          for p in range(npair):
                              sps = pbs.tile([128, 1024], F32, tag="s")
                              diag = False
                              for half in range(2):
                                  idx = 2 * p + half
                                  jt = jts[idx]
                                  nc.tensor.matmul(
                                      sps[:, half * 512:(half + 1) * 512],
                                      kT_sb[:, jt * 128:(jt + 1) * 128],
                                      qh[:, isl],
                                      start=True,
                                      stop=True,
                                  )
                                  if mode == "causal" and jt >= 4 * ig:
                                      diag = True
                                  if idx >= 4:
                                      emit_pv(idx - 4)
                                  if half == 1 and ci < len(chunks):
                                      emit_cchunk(st_prev, chunks[ci])
                                      ci += 1
                              e = pe_pool.tile([128, 1024], BF16, tag="E", name=f"E{p}")
                              et.append(e)
                              if masked:
                                  # host pre-scales maskT by sqrt(HD):
                                  # exp(SCALE*(scores + maskT)) == softmax logits
                                  for half in range(2):
                                      jt = jts[2 * p + half]
                                      hsl = slice(half * 512, (half + 1) * 512)
                                      mt = pbm.tile([128, 512], F32, tag="mT")
                                      nc.sync.dma_start(
                                          mt[:], maskT[jt * 128:(jt + 1) * 128, isl]
                                      )
                                      sm = pbm.tile([128, 512], F32, tag="sm")
                                      nc.vector.tensor_add(sm[:], sps[:, hsl], mt[:])
                                      nc.scalar.activation(
                                          e[:, hsl], sm[:],
                                          mybir.ActivationFunctionType.Exp,
                                          scale=SCALE,
                                      )
                              elif diag:
                                  # pair contains diagonal tiles: exp then zero
                                  # the j>i part per half
                                  for half in range(2):
                                      jt = jts[2 * p + half]
                                      hsl = slice(half * 512, (half + 1) * 512)
                                      if jt >= 4 * ig:
                                          dp = jt - 4 * ig
                                          etmp = pbm.tile([128, 512], BF16, tag="etmp")
                                          nc.scalar.activation(
                                              etmp[:], sps[:, hsl],
                                              mybir.ActivationFunctionType.Exp,
                                              scale=SCALE,
                                          )
                                          nc.vector.tensor_mul(
                                              e[:, hsl], etmp[:],
                                              m01_sb[:, dp * 512:(dp + 1) * 512],
                                          )
                                      else:
                                          nc.scalar.activation(
                                              e[:, hsl], sps[:, hsl],
                                              mybir.ActivationFunctionType.Exp,
                                              scale=SCALE,
                                          )
                              else:
                                  # fused: one activation covers both halves
                                  nc.scalar.activation(
                                      e[:], sps[:],
                                      mybir.ActivationFunctionType.Exp,
                                      scale=SCALE,
                                  )
                              # denominator: pair-sum then running-sum chain,
                              # emitted inline so the adds run as exps land.
                              # The last pair stays off the Pool engine (its
                              # ~1.2us add would sit on the critical tail).
                              a = pacc.tile([128, 512], BF16, tag="acc", name=f"A{p}")
                              eng = nc.gpsimd if (p in (1, 3, 5) and p != npair - 1) else nc.vector
                              eng.tensor_add(a[:], e[:, 0:512], e[:, 512:1024])
                              if T is None:
                                  T = a
                              else:
                                  Tn = pacc.tile([128, 512], BF16, tag="acc", name=f"T{p}")
                                  nc.vector.tensor_add(Tn[:], T[:], a[:])
                                  T = Tn
                          while ci < len(chunks):
                              emit_cchunk(st_prev, chunks[ci])
                              ci += 1
                          for idx in range(max(0, n - 4), n):
                              emit_pv(idx)
                          den = pbd.tile([128, 512], F32, tag="den")
                          nc.tensor.matmul(
                              den[:], ones_sb[:], T[:], start=True, stop=True
                          )
                          rc = pbt.tile([128, 512], F32, tag="rc")
                          nc.vector.reciprocal_approx_fast(rc[:], den[:])
                          nc.vector.tensor_mul(aT_sb[h][:, isl], pv[:], rc[:])

                  # trailing output-projection blocks for the last query group
                  for h in range(QH):
                      st = 4 * (SG - 1) + h
                      for ho in range(H // 512):
                          emit_cchunk(st, ho)

    nc.finalize()
    return nc


def _get_kernel(mode: str):
    if mode not in _BUILT:
        _BUILT[mode] = _build(mode)
    return _BUILT[mode]


def _detect_mode(mask2d):
    if not np.any(mask2d):
        return "nomask"
    neg = mask2d[0, 1]
    if neg <= -1e4 and np.array_equal(
        mask2d, np.triu(np.full((S, S), neg, mask2d.dtype), k=1)
    ):
        return "causal"
    return "generic"


def kernel(hidden_states, position_ids, attention_mask, cos, sin, Wq, Wk, Wv, Wo,
           _collect_exec_info=None):
    hidden_states = np.asarray(hidden_states)
    attention_mask = np.asarray(attention_mask)
    cos = np.asarray(cos)
    sin = np.asarray(sin)
    Wq, Wk, Wv, Wo = (np.asarray(a) for a in (Wq, Wk, Wv, Wo))

    mode = _detect_mode(attention_mask[0, 0])
    masked = mode == "generic"
    nc = _get_kernel(mode)

    hT = np.ascontiguousarray(hidden_states[0].T).astype(NPBF16)
    cosT = np.ascontiguousarray(cos[0].T).astype(np.float32)
    sinTe = np.ascontiguousarray(sin[0].T).astype(np.float32)
    sinTe[:64] = -sinTe[:64]
    sinTe = np.ascontiguousarray(np.roll(sinTe, 64, axis=0))
    eye = np.eye(128, dtype=NPBF16)
    if mode == "causal":
        jj = np.arange(128)[:, None]
        ii = np.arange(512)[None, :]
        m01 = np.concatenate(
            [(128 * p + jj <= ii).astype(NPBF16) for p in range(4)], axis=0
        )

    in_maps = []
    for c in range(N_CORES):
        wqkv = np.concatenate(
            [
                Wq[:, c * F:(c + 1) * F],
                Wk[:, c * HD:(c + 1) * HD],
                Wv[:, c * HD:(c + 1) * HD],
            ],
            axis=1,
        ).astype(NPBF16)
        m = {
            "hT": hT,
            "wqkv": wqkv,
            "wo": Wo[c * F:(c + 1) * F, :].astype(NPBF16),
            "cosT": cosT,
            "sinTe": sinTe,
            "eye": eye,
        }
        if masked:
            m["maskT"] = (
                np.ascontiguousarray(attention_mask[0, 0].T).astype(np.float32)
                * math.sqrt(HD)
            )
        if mode == "causal":
            m["m01"] = m01
        in_maps.append(m)

    trace = _collect_exec_info is not None
    res = run_bass_kernel_spmd(nc, in_maps, list(range(N_CORES)), trace=trace)
    if trace:
        _collect_exec_info["exec_time_ns"] = res.exec_time_ns
        _collect_exec_info["results"] = res

    acc = res.results[0]["out"].astype(np.float64)
    for c in range(1, N_CORES):
        acc += res.results[c]["out"].astype(np.float64)
    return acc.astype(np.float32)[None, :, :]


# revision 10
# speedup vs baseline: 1.2592x; 1.0963x over previous
"""Multi-head attention (GQA, 32 q-heads / 8 kv-heads, S=2048, H=4096) on 8
Trainium2 NeuronCores.

Sharding: tensor-parallel across heads. Core c owns kv-head c and q-heads
4c..4c+3 (Wq/Wk/Wv column-sharded, Wo row-sharded). Each core computes a
partial output [S, H] in fp16; the host sums the 8 partials.

Per-core dataflow (everything bf16 into the PE, fp32 accumulation):
  A) qT/kT/vT = W.T @ hiddenT  (weights stationary, hiddenT moving)
     + RoPE applied in the transposed [hd, s] layout
     + vT transposed back to natural v[s, hd] via PE-transpose
     sg0/sg1 run k-major (DMA-paced); sg2/sg3 run f-major so PSUM
     evictions hide under the next feature group's matmuls.
  B) per (q-group ig, head h): scoresT[j,i] = kT.T @ qT -> E = exp(...)
     denom via a bf16 add-tree over the 16 E tiles (DVE+Pool) and a
     single ones-matmul; attnT[d,i] = v.T @ E, normalized on DVE.
  C) partial_out[s,:] = attnT.T @ Wo_c, interleaved INTO phase B: the
     C-block for query-group ig-1 is issued between the scores matmuls
     of group ig so the PE stays fed while ACT drains the exps.
"""

import math
import os
import sys

if os.path.isdir("/opt/trn_rl_repo") and "/opt/trn_rl_repo" not in sys.path:
    sys.path.insert(0, "/opt/trn_rl_repo")

import numpy as np
import ml_dtypes

import concourse.bacc as bacc
import concourse.mybir as mybir
from concourse import tile
from concourse.bass_utils import run_bass_kernel_spmd

BF16 = mybir.dt.bfloat16
F16 = mybir.dt.float16
F32 = mybir.dt.float32
NPBF16 = ml_dtypes.bfloat16

S = 2048
H = 4096
HD = 128
NH = 32
NKV = 8
N_CORES = 8
QH = NH // N_CORES          # q-heads per core = 4
F = QH * HD                 # q feature columns per core = 512
KT = H // 128               # contraction tiles for the projections = 32
ST = S // 128               # 128-row tiles along S = 16
SG = S // 512               # 512-wide groups along S = 4
SCALE = 1.0 / math.sqrt(HD)

_BUILT = {}

# feature-group order: k first (phase B needs kT complete first), then v,
# then the q heads
ORDER = (4, 5, 0, 1, 2, 3)


def _build(mode: str):
    masked = mode == "generic"
    nc = bacc.Bacc(None, target_bir_lowering=False)

    hT = nc.declare_dram_parameter("hT", [H, S], BF16, isOutput=False)
    wqkv = nc.declare_dram_parameter("wqkv", [H, F + 2 * HD], BF16, isOutput=False)
    wo = nc.declare_dram_parameter("wo", [F, H], BF16, isOutput=False)
    cosT = nc.declare_dram_parameter("cosT", [HD, S], F32, isOutput=False)
    sinTe = nc.declare_dram_parameter("sinTe", [HD, S], F32, isOutput=False)
    eye = nc.declare_dram_parameter("eye", [128, 128], BF16, isOutput=False)
    if masked:
        maskT = nc.declare_dram_parameter("maskT", [S, S], F32, isOutput=False)
    if mode == "causal":
        # four 0/1 diagonal-tile patterns, stacked [4*128, 512]
        m01 = nc.declare_dram_parameter("m01", [4 * 128, 512], BF16, isOutput=False)
    out = nc.declare_dram_parameter("out", [S, H], F16, isOutput=True)

    FW = F + 2 * HD  # 768 weight columns per contraction tile

    with tile.TileContext(nc) as tc:
        with tc.tile_pool(name="persist", bufs=1) as pp:
            # persistent SBUF tensors
            cos_sb = pp.tile([HD, S], F32, tag="cos")
            sin_sb = pp.tile([HD, S], F32, tag="sin")
            eye_sb = pp.tile([128, 128], BF16, tag="eye")
            ones_sb = pp.tile([128, 128], BF16, tag="ones")
            qT_sb = [pp.tile([HD, S], BF16, tag=f"qT{h}", name=f"qT{h}") for h in range(QH)]
            kT_sb = pp.tile([HD, S], BF16, tag="kT")
            v_sb = pp.tile([128, ST * HD], BF16, tag="v")  # block jt: v[jt*128:(jt+1)*128, :]
            aT_sb = [pp.tile([HD, S], BF16, tag=f"aT{h}", name=f"aT{h}") for h in range(QH)]
            # RoPE scratch lives in the persistent pool so phase-B tiles
            # never alias it
            vt_t = pp.tile([128, 512], BF16, tag="vt")
            ev_t = {
                f: pp.tile([128, 512], F32, tag=f"ev{f}", name=f"ev{f}")
                for f in (4, 0, 1, 2, 3)
            }
            t1_t = pp.tile([128, 512], F32, tag="t1")
            t2_t = pp.tile([128, 512], F32, tag="t2")
            if mode == "causal":
                m01_sb = pp.tile([128, 4 * 512], BF16, tag="m01")

            nc.gpsimd.memset(ones_sb[:], 1.0)

            # wqkv column ranges per feature group
            def wcol(f):
                base = f * 128 if f < QH else F + (f - QH) * HD
                return base

            # ---------------- Phase A: projections + RoPE ----------------
            with (
                tc.tile_pool(name="phA_sb", bufs=1) as pa,
                tc.tile_pool(name="phA_h", bufs=2) as pah,
                tc.tile_pool(name="phA_ps", bufs=1, space="PSUM") as pap,
                tc.tile_pool(name="phA_pst", bufs=2, space="PSUM") as papt,
            ):
                w_sb = pa.tile([128, KT * FW], BF16, tag="wqkv")
                w_view = w_sb[:].rearrange("p (a f) -> p a f", a=KT)
                w_src = wqkv[:].rearrange("(a p) f -> p a f", p=128)
                CH = 4  # ktiles per DMA chunk

                pending_tr = []

                def flush_tr():
                    for fn in pending_tr:
                        fn()
                    pending_tr.clear()

                def evict_f(sg, f, pss_f):
                    """PSUM -> SBUF eviction + RoPE (or v transpose) for one
                    feature group."""
                    sl = slice(sg * 512, (sg + 1) * 512)
                    if f == 5:
                        # v: copy out now; the PE transposes back to natural
                        # [s, hd] layout are deferred until after the next
                        # matmul group so the PE never waits on the copy
                        nc.vector.tensor_copy(vt_t[:], pss_f[:])

                        def tr(sg=sg):
                            for b in range(4):
                                jt = sg * 4 + b
                                pst = papt.tile([128, 128], BF16, tag="vtr")
                                nc.tensor.transpose(
                                    pst[:], vt_t[:, b * 128:(b + 1) * 128], eye_sb[:]
                                )
                                cp = nc.vector.tensor_copy if b % 2 == 0 else nc.scalar.copy
                                cp(v_sb[:, jt * HD:(jt + 1) * HD], pst[:])

                        pending_tr.append(tr)
                        return
                    ev = ev_t[f]
                    if f in (0, 2):
                        nc.vector.tensor_copy(ev[:], pss_f[:])
                    else:
                        nc.scalar.copy(ev[:], pss_f[:])
                    # RoPE: out[d] = x[d]*cos[d] + x[(d+64)%128]*sinTe[d]
                    dest = (qT_sb[f] if f < QH else kT_sb)[:, sl]
                    nc.vector.tensor_mul(t1_t[:], ev[:], cos_sb[:, sl])
                    # sin table is host-rolled by 64 rows so both SBUF
                    # inputs share a base partition (walrus constraint)
                    nc.vector.tensor_mul(
                        t2_t[0:64, :], ev[64:128, :], sin_sb[64:128, sl]
                    )
                    nc.vector.tensor_mul(
                        t2_t[64:128, :], ev[0:64, :], sin_sb[0:64, sl]
                    )
                    nc.vector.tensor_add(dest, t1_t[:], t2_t[:])

                for sg in range(SG):
                    hc = pah.tile([128, KT * 512], BF16, tag="hc")
                    h_view = hc[:].rearrange("p (a s) -> p a s", a=KT)
                    h_src = hT[:, sg * 512:(sg + 1) * 512].rearrange(
                        "(a p) s -> p a s", p=128
                    )
                    if sg == 0:
                        # weights stream on the sync queue, hidden chunks on
                        # the scalar queue: the two transfers run in parallel
                        # and the PE can start as soon as k-tile 0 lands.
                        # First weight piece is just the kv columns of k-tile
                        # 0 -- the very first matmul group (f=4) needs only
                        # those.
                        nc.sync.dma_start(w_view[:, 0:1, F:FW], w_src[:, 0:1, F:FW])
                        nc.scalar.dma_start(h_view[:, 0:1, :], h_src[:, 0:1, :])
                        nc.sync.dma_start(w_view[:, 0:1, 0:F], w_src[:, 0:1, 0:F])
                        bounds = sorted(set([1, 2, 3, 4, 6] + list(range(8, KT, CH)) + [KT]))
                        for lo, hi in zip(bounds[:-1], bounds[1:]):
                            csl = slice(lo, hi)
                            nc.sync.dma_start(w_view[:, csl, :], w_src[:, csl, :])
                            nc.scalar.dma_start(h_view[:, csl, :], h_src[:, csl, :])
                            if lo == 28:
                                # needed only ~40us in; keep off the queue head
                                nc.sync.dma_start(eye_sb[:], eye[:])
                                nc.sync.dma_start(cos_sb[:], cosT[:])
                                nc.sync.dma_start(sin_sb[:], sinTe[:])
                                if mode == "causal":
                                    nc.sync.dma_start(
                                        m01_sb[:].rearrange("p (a i) -> p a i", a=4),
                                        m01[:].rearrange("(a p) i -> p a i", p=128),
                                    )
                    else:
                        # chunked so early k-tiles unblock as they land (a
                        # single dma_start only signals at full completion)
                        for lo in range(0, KT, 8):
                            nc.scalar.dma_start(
                                h_view[:, lo:lo + 8, :], h_src[:, lo:lo + 8, :]
                            )

                    pss = {
                        f: pap.tile([128, 512], F32, tag=f"proj{f}", name=f"proj{f}")
                        for f in ORDER
                    }
                    if sg < 2:
                        # k-major: tolerant of streaming DMA arrival
                        for k in range(KT):
                            for f in ORDER:
                                nc.tensor.matmul(
                                    pss[f][:],
                                    w_sb[:, k * FW + wcol(f):k * FW + wcol(f) + 128],
                                    hc[:, k * 512:(k + 1) * 512],
                                    start=(k == 0),
                                    stop=(k == KT - 1),
                                )
                        for f in ORDER:
                            # flush the previous sg's deferred v-transposes
                            # before this sg's v eviction reuses vt_t
                            if f == 5:
                                flush_tr()
                            evict_f(sg, f, pss[f])
                    else:
                        # f-major: evictions hide under the next group
                        for f in ORDER:
                            for k in range(KT):
                                nc.tensor.matmul(
                                    pss[f][:],
                                    w_sb[:, k * FW + wcol(f):k * FW + wcol(f) + 128],
                                    hc[:, k * 512:(k + 1) * 512],
                                    start=(k == 0),
                                    stop=(k == KT - 1),
                                )
                            flush_tr()
                            evict_f(sg, f, pss[f])
                flush_tr()

            # ------------- Phase B+C: attention + output projection -------------
            with tc.tile_pool(name="late", bufs=1) as pl:
              # Wo transfer hides under early phase B compute
              wo_sb = pl.tile([128, QH * H], BF16, tag="wo")
              nc.sync.dma_start(
                  wo_sb[:].rearrange("p (a o) -> p a o", a=QH),
                  wo[:].rearrange("(a p) o -> p a o", p=128),
              )
              with (
                tc.tile_pool(name="phB_E", bufs=18) as pe_pool,
                tc.tile_pool(name="phB_acc", bufs=16) as pacc,
                tc.tile_pool(name="phB_tmp", bufs=3) as pbt,
                tc.tile_pool(name="phB_m", bufs=3) as pbm,
                tc.tile_pool(name="phB_ob", bufs=4) as pco,
                tc.tile_pool(name="phB_s", bufs=2, space="PSUM") as pbs,
                tc.tile_pool(name="phB_pv", bufs=1, space="PSUM") as pbv,
                tc.tile_pool(name="phB_den", bufs=1, space="PSUM") as pbd,
                tc.tile_pool(name="phB_po", bufs=2, space="PSUM") as pcp,
              ):
                  def emit_cchunk(st, ho):
                      """One (st, ho) block of the output projection:
                      out[st*128:(st+1)*128, ho*512:(ho+1)*512]."""
                      ssl = slice(st * 128, (st + 1) * 128)
                      po = pcp.tile([128, 512], F32, tag="po")
                      for f4 in range(QH):
                          nc.tensor.matmul(
                              po[:],
                              aT_sb[f4][:, ssl],
                              wo_sb[:, f4 * H + ho * 512:f4 * H + (ho + 1) * 512],
                              start=(f4 == 0),
                              stop=(f4 == QH - 1),
                          )
                      ob = pco.tile([128, 512], F16, tag="ob")
                      cp = nc.vector.tensor_copy if ho % 2 == 0 else nc.scalar.copy
                      cp(ob[:], po[:])
                      nc.sync.dma_start(out[ssl, ho * 512:(ho + 1) * 512], ob[:])

                  for ig in range(SG):
                      isl = slice(ig * 512, (ig + 1) * 512)
                      # causal: key tiles past this query block contribute
                      # exactly zero -- skip them entirely
                      jts = list(range(4 * ig + 4)) if mode == "causal" else list(range(ST))
                      n = len(jts)
                      npair = n // 2
                      for h in range(QH):
                          qh = qT_sb[h]
                          st_prev = 4 * (ig - 1) + h
                          chunks = list(range(H // 512)) if ig > 0 else []
                          ci = 0
                          et = []
                          pv = pbv.tile([128, 512], F32, tag="pv")
                          T = None  # running denominator partial sum

                          def emit_pv(idx):
                              jt = jts[idx]
                              src = et[idx // 2][:, (idx % 2) * 512:(idx % 2 + 1) * 512]
                              nc.tensor.matmul(
                                  pv[:], v_sb[:, jt * HD:(jt + 1) * HD], src,
                                  start=(idx == 0), stop=(idx == n - 1),
                              )

                          for p in range(npair):
                              sps = pbs.tile([128, 1024], F32, tag="s")
                              diag = False
                              for half in range(2):
                                  idx = 2 * p + half
                                  jt = jts[idx]
                                  nc.tensor.matmul(
                                      sps[:, half * 512:(half + 1) * 512],
                                      kT_sb[:, jt * 128:(jt + 1) * 128],
                                      qh[:, isl],
                                      start=True,
                                      stop=True,
                                  )
                                  if mode == "causal" and jt >= 4 * ig:
                                      diag = True
                                  if idx >= 4:
                                      emit_pv(idx - 4)
                                  if half == 1 and ci < len(chunks):
                                      emit_cchunk(st_prev, chunks[ci])
                                      ci += 1
                              e = pe_pool.tile([128, 1024], BF16, tag="E", name=f"E{p}")
                              et.append(e)
                              if masked:
                                  # host pre-scales maskT by sqrt(HD):
                                  # exp(SCALE*(scores + maskT)) == softmax logits
                                  for half in range(2):
                                      jt = jts[2 * p + half]
                                      hsl = slice(half * 512, (half + 1) * 512)
                                      mt = pbm.tile([128, 512], F32, tag="mT")
                                      nc.sync.dma_start(
                                          mt[:], maskT[jt * 128:(jt + 1) * 128, isl]
                                      )
                                      sm = pbm.tile([128, 512], F32, tag="sm")
                                      nc.vector.tensor_add(sm[:], sps[:, hsl], mt[:])
                                      nc.scalar.activation(
                                          e[:, hsl], sm[:],
                                          mybir.ActivationFunctionType.Exp,
                                          scale=SCALE,
                                      )
                              elif diag:
                                  # pair contains diagonal tiles: exp then zero
                                  # the j>i part per half
                                  for half in range(2):
                                      jt = jts[2 * p + half]
                                      hsl = slice(half * 512, (half + 1) * 512)
                                      if jt >= 4 * ig:
                                          dp = jt - 4 * ig
                                          etmp = pbm.tile([128, 512], BF16, tag="etmp")
                                          nc.scalar.activation(
                                              etmp[:], sps[:, hsl],
                                              mybir.ActivationFunctionType.Exp,
                                              scale=SCALE,
                                          )
                                          nc.vector.tensor_mul(
                                              e[:, hsl], etmp[:],
                                              m01_sb[:, dp * 512:(dp + 1) * 512],
                                          )
                                      else:
                                          nc.scalar.activation(
                                              e[:, hsl], sps[:, hsl],
                                              mybir.ActivationFunctionType.Exp,
                                              scale=SCALE,
                                          )
                              else:
                                  # fused: one activation covers both halves
                                  nc.scalar.activation(
                                      e[:], sps[:],
                                      mybir.ActivationFunctionType.Exp,
                                      scale=SCALE,
                                  )
                              # denominator: pair-sum then running-sum chain,
                              # emitted inline so the adds run as exps land.
                              # The last pair stays off the Pool engine (its
                              # ~1.2us add would sit on the critical tail).
                              a = pacc.tile([128, 512], BF16, tag="acc", name=f"A{p}")
                              eng = nc.gpsimd if (p in (1, 3, 5) and p != npair - 1) else nc.vector
                              eng.tensor_add(a[:], e[:, 0:512], e[:, 512:1024])
                              if T is None:
                                  T = a
                              else:
                                  Tn = pacc.tile([128, 512], BF16, tag="acc", name=f"T{p}")
                                  nc.vector.tensor_add(Tn[:], T[:], a[:])
                                  T = Tn
                          while ci < len(chunks):
                              emit_cchunk(st_prev, chunks[ci])
                              ci += 1
                          den = pbd.tile([128, 512], F32, tag="den")
                          last_step = ig == SG - 1 and h == QH - 1
                          if last_step:
                              # start the reciprocal chain early: the trailing
                              # output-projection blocks wait on this head's aT
                              nc.tensor.matmul(
                                  den[:], ones_sb[:], T[:], start=True, stop=True
                              )
                          for idx in range(max(0, n - 4), n):
                              emit_pv(idx)
                          if not last_step:
                              nc.tensor.matmul(
                                  den[:], ones_sb[:], T[:], start=True, stop=True
                              )
                          rc = pbt.tile([128, 512], F32, tag="rc")
                          nc.vector.reciprocal_approx_fast(rc[:], den[:])
                          nc.vector.tensor_mul(aT_sb[h][:, isl], pv[:], rc[:])

                  # trailing output-projection blocks for the last query group
                  for h in range(QH):
                      st = 4 * (SG - 1) + h
                      for ho in range(H // 512):
                          emit_cchunk(st, ho)

    nc.finalize()
    return nc


def _get_kernel(mode: str):
    if mode not in _BUILT:
        _BUILT[mode] = _build(mode)
    return _BUILT[mode]


def _detect_mode(mask2d):
    if not np.any(mask2d):
        return "nomask"
    neg = mask2d[0, 1]
    if neg <= -1e4 and np.array_equal(
        mask2d, np.triu(np.full((S, S), neg, mask2d.dtype), k=1)
    ):
        return "causal"
    return "generic"


def kernel(hidden_states, position_ids, attention_mask, cos, sin, Wq, Wk, Wv, Wo,
           _collect_exec_info=None):
    hidden_states = np.asarray(hidden_states)
    attention_mask = np.asarray(attention_mask)
    cos = np.asarray(cos)
    sin = np.asarray(sin)
    Wq, Wk, Wv, Wo = (np.asarray(a) for a in (Wq, Wk, Wv, Wo))

    mode = _detect_mode(attention_mask[0, 0])
    masked = mode == "generic"
    nc = _get_kernel(mode)

    hT = np.ascontiguousarray(hidden_states[0].T).astype(NPBF16)
    cosT = np.ascontiguousarray(cos[0].T).astype(np.float32)
    sinTe = np.ascontiguousarray(sin[0].T).astype(np.float32)
    sinTe[:64] = -sinTe[:64]
    sinTe = np.ascontiguousarray(np.roll(sinTe, 64, axis=0))
    eye = np.eye(128, dtype=NPBF16)
    if mode == "causal":
        jj = np.arange(128)[:, None]
        ii = np.arange(512)[None, :]
        m01 = np.concatenate(
            [(128 * p + jj <= ii).astype(NPBF16) for p in range(4)], axis=0
        )

    in_maps = []
    for c in range(N_CORES):
        wqkv = np.concatenate(
            [
                Wq[:, c * F:(c + 1) * F],
                Wk[:, c * HD:(c + 1) * HD],
                Wv[:, c * HD:(c + 1) * HD],
            ],
            axis=1,
        ).astype(NPBF16)
        m = {
            "hT": hT,
            "wqkv": wqkv,
            "wo": Wo[c * F:(c + 1) * F, :].astype(NPBF16),
            "cosT": cosT,
            "sinTe": sinTe,
            "eye": eye,
        }
        if masked:
            m["maskT"] = (
                np.ascontiguousarray(attention_mask[0, 0].T).astype(np.float32)
                * math.sqrt(HD)
            )
        if mode == "causal":
            m["m01"] = m01
        in_maps.append(m)

    trace = _collect_exec_info is not None
    res = run_bass_kernel_spmd(nc, in_maps, list(range(N_CORES)), trace=trace)
    if trace:
        _collect_exec_info["exec_time_ns"] = res.exec_time_ns
        _collect_exec_info["results"] = res

    acc = res.results[0]["out"].astype(np.float64)
    for c in range(1, N_CORES):
        acc += res.results[c]["out"].astype(np.float64)
    return acc.astype(np.float32)[None, :, :]
